# revision 24
# baseline (speedup 1.0000x reference)
"""Linear-chain CRF forward loss on 8 Trainium2 NeuronCores.

Math: per (channel, batch) row the reference runs a T=2048-step log-space
scan  alpha_t[j] = logsumexp_i(alpha_{t-1}[i] + trans[i,j]) + em_t[j]  and
returns -(z_sup - z_full).  Rewritten in linear space:

    S_k = (E'^T S_{k-1}) * X'_k        (elementwise in X')

with E' = exp(trans, forbidden->0)/128 in bf16 and X'[k][j,row] =
exp(em - sh)*128 in fp8-e4m3 (channel-0 rows masked by target), where sh is
a per-(t,row) host-side log-growth normalizer added back at the end.  The
*128//128 scaling centres X' in fp8's normal range and cancels exactly.

Sharding: T is split into 256 chunks of 8 steps (32 chains per core).
Products of positive matrices contract to rank-1 so fast that a chunk
started from the raw direction X'[t0] already telescopes correctly with NO
on-chip warm-up: the chunk-start column sums are computed on the host
(they are just colsum(X'[t0]); the pinned last chunk replays its 1 warm-up
step in emulated bf16/fp8 host arithmetic).  Per core the 32 chains run as
4 groups of 8: each round does two [128x128x512] bf16 matmuls per group
into a 2-bank PSUM tile, then the elementwise multiply is split across
three engines - DVE multiplies columns [0:672] straight out of PSUM, ACT
copies columns [672:1024] to SBUF bf16, and GPSIMD multiplies those in its
16-bit path.  End-of-chunk column sums (ones|exp(end) stationary matmul
probes) are DMA'd out and telescoped on the host in f64.
"""

import math

import numpy as np
import ml_dtypes

import concourse.bacc as bacc
import concourse.mybir as mybir
import concourse.tile as tile
from concourse.bass_utils import run_bass_kernel_spmd

B, T, N = 64, 2048, 128
R = 2 * B
NCORES = 8
G = 4                      # chain groups per core
K = 8                      # chains per group (fused width K*R = 1024)
NCHUNK = NCORES * G * K    # 256
W = 0                      # no on-chip warm-up; host computes start colsums
L = math.ceil((T + (NCHUNK - 1) * W) / NCHUNK)          # 8
KSTAR = (L - W) * (NCHUNK - 2) + L - (T - 1 - L)        # 1 (last chunk)
XTIERS = [1, 1, 2, 4]      # X prefetch tier sizes (steps); sum == L
FR = K * R                 # fused row width (1024)
HFR = FR // 2              # matmul moving-dim max is 512
ASPL = 672                 # columns multiplied by DVE direct from PSUM
BSPL = FR - ASPL           # columns via ACT copy -> GPSIMD multiply

F32 = mybir.dt.float32
BF16 = mybir.dt.bfloat16
FP8 = mybir.dt.float8e4

NP_BF16 = ml_dtypes.bfloat16
NP_FP8 = ml_dtypes.float8_e4m3

_COMPILED = {}


def _build_nc():
    if "nc" in _COMPILED:
        return _COMPILED["nc"]

    assert sum(XTIERS) == L
    nc = bacc.Bacc("TRN2", target_bir_lowering=False, debug=False,
                   num_devices=NCORES)

    e_d = nc.dram_tensor("e", [N, N], BF16, kind="ExternalInput").ap()
    oe_d = nc.dram_tensor("oe", [N, 2], BF16, kind="ExternalInput").ap()
    x_d = nc.dram_tensor("x", [N, L, G, FR], FP8,
                         kind="ExternalInput").ap()
    i_ds = [nc.dram_tensor(f"i{g}", [N, FR], BF16,
                           kind="ExternalInput").ap() for g in range(G)]
    out_d = nc.dram_tensor("probes", [G, 2, FR], F32,
                           kind="ExternalOutput").ap()

    with tile.TileContext(nc) as tc:
        with (
            tc.tile_pool(name="consts", bufs=1) as consts,
            tc.tile_pool(name="states", bufs=3) as states,
            tc.tile_pool(name="qbstage", bufs=2) as qbstage,
            tc.tile_pool(name="xtiles", bufs=1) as xtiles,
            tc.tile_pool(name="stage", bufs=1) as stage,
            tc.tile_pool(name="qpsum", bufs=1, space="PSUM") as qpsum,
        ):
            # dummy ops pull the ACT table load / GPSIMD lib load into the
            # program preamble instead of blocking the first real round
            dum = consts.tile([1, 4], BF16, tag="dum")
            nc.scalar.copy(out=dum[:, 2:4], in_=dum[:, 0:2])
            nc.gpsimd.tensor_mul(out=dum[:, 2:4], in0=dum[:, 0:2],
                                 in1=dum[:, 0:2])

            # Transfers queue in-order per HWDGE ring, so the SP ring gets
            # the small round-0 dependencies (e, inits) plus the late bulk
            # tiers, while the ACT ring moves the early X tiers in
            # parallel.  Nothing issues from the compute-critical Pool.
            xtile = []
            tier_of = []
            lo = 0
            for t, nb in enumerate(XTIERS):
                tier_of += [(t, lo)] * nb
                lo += nb

            # round-0 deps (e, inits) on the SP ring; early X tiers on the
            # ACT ring so the transfers proceed in parallel; bulk after.
            e_sb = consts.tile([N, N], BF16)
            nc.sync.dma_start(out=e_sb, in_=e_d)
            S = []
            for g in range(G):
                sg = states.tile([N, FR], BF16, tag=f"s{g}")
                nc.sync.dma_start(out=sg, in_=i_ds[g])
                S.append(sg)
            # alternate tiers across the two HWDGE rings so consecutive
            # tiers transfer in parallel instead of queueing
            for t in range(len(XTIERS)):
                nb = XTIERS[t]
                lo = sum(XTIERS[:t])
                xt = xtiles.tile([N, nb, G, FR], FP8, tag=f"t{t}")
                eng = nc.scalar if t % 2 == 0 else nc.sync
                eng.dma_start(out=xt, in_=x_d[:, lo:lo + nb, :, :])
                xtile.append(xt)
            oe_sb = consts.tile([N, 2], BF16)
            nc.sync.dma_start(out=oe_sb, in_=oe_d)  # needed only at k=L

            for k in range(1, L + 1):
                t, lo = tier_of[k - 1]
                for g in range(G):
                    q = qpsum.tile([N, FR], F32, tag=f"q{g}")
                    nc.tensor.matmul(q[:, :HFR], lhsT=e_sb,
                                     rhs=S[g][:, :HFR],
                                     start=True, stop=True)
                    nc.tensor.matmul(q[:, HFR:], lhsT=e_sb,
                                     rhs=S[g][:, HFR:],
                                     start=True, stop=True)
                    s_new = states.tile([N, FR], BF16, tag=f"s{g}")
                    xk = xtile[t][:, k - 1 - lo, g, :]
                    # DVE: columns [0:ASPL] straight from PSUM (2-bank span)
                    nc.vector.tensor_mul(out=s_new[:, :ASPL],
                                         in0=q[:, :ASPL], in1=xk[:, :ASPL])
                    # ACT copies [ASPL:] to bf16, GPSIMD multiplies
                    qb = qbstage.tile([N, BSPL], BF16, tag=f"qb{g}")
                    nc.scalar.copy(out=qb, in_=q[:, ASPL:])
                    nc.gpsimd.tensor_mul(out=s_new[:, ASPL:], in0=qb,
                                         in1=xk[:, ASPL:])
                    S[g] = s_new

            # end-of-chunk probes: colsum and exp(end)-dot of S(L), reusing
            # each group's PSUM banks (partitions 0:2)
            for g in range(G):
                p = qpsum.tile([N, FR], F32, tag=f"q{g}")
                nc.tensor.matmul(p[0:2, :HFR], lhsT=oe_sb,
                                 rhs=S[g][:, :HFR], start=True, stop=True)
                nc.tensor.matmul(p[0:2, HFR:], lhsT=oe_sb,
                                 rhs=S[g][:, HFR:], start=True, stop=True)
                st = stage.tile([2, FR], F32, tag=f"st{g}")
                nc.scalar.copy(out=st, in_=p[0:2, :])
                nc.sync.dma_start(out=out_d[g], in_=st)

    nc.compile()
    _COMPILED["nc"] = nc
    return nc


def _host_prep(inputs):
    em = np.asarray(inputs["emissions"], np.float32)
    tgt = np.asarray(inputs["target"])
    trans = np.asarray(inputs["transitions"], np.float32)
    st = np.asarray(inputs["start_transitions"], np.float32)
    en = np.asarray(inputs["end_transitions"], np.float32)
    ft = np.asarray(inputs["forbidden_transitions"]).astype(bool)
    sft = np.asarray(inputs["start_forbidden_transitions"]).astype(bool)
    eft = np.asarray(inputs["end_forbidden_transitions"]).astype(bool)
    mask = np.asarray(inputs["mask"]).astype(bool)
    assert mask.all(), "kernel specialized for all-true mask"

    E = np.where(ft, 0.0, np.exp(trans)).astype(np.float32)
    expst = np.where(sft, 0.0, np.exp(st)).astype(np.float32)
    expen = np.where(eft, 0.0, np.exp(en)).astype(np.float32)

    x1 = np.exp(em.astype(np.float32)).transpose(1, 2, 0)    # [T,N,B]
    x0 = x1 * tgt.astype(np.float32).transpose(1, 2, 0)
    X = np.concatenate([x0, x1], axis=2)                     # [T,N,R]

    Ebar = np.float64(E.astype(np.float64).mean())
    sh = np.log(np.maximum(X.sum(axis=1, dtype=np.float64) * Ebar, 1e-300))
    Xs = (X * (np.exp(-sh)[:, None, :] * 128.0)).astype(np.float32)
    Xq = np.minimum(Xs, np.float32(240.0)).astype(NP_FP8)    # [T,N,R] fp8
    return E, expst, expen, Xq, sh


def kernel(**inputs):
    loss, _ = _run(inputs)
    return loss


def _to_bf16(x):
    """Round-to-nearest-even f32 -> bf16 -> f32 (matches device rounding)."""
    u = np.ascontiguousarray(x, np.float32).view(np.uint32)
    r = (u + 0x7FFF + ((u >> 16) & 1)) & 0xFFFF0000
    return r.view(np.float32)


def _host_start_colsums(e_in, Xq, t0s):
    """Chunk-start column sums, host side.  Mid chunks start at the raw
    direction X'[t0] (W=0), so the start colsum is just its column sum.
    The pinned last chunk replays its KSTAR warm-up steps in emulated
    device arithmetic (bf16 matmul inputs, f32 accumulate, bf16 state)."""
    csW = np.stack([Xq[t0s[j]].astype(np.float64).sum(axis=0)
                    for j in range(1, NCHUNK - 1)])          # [NCHUNK-2, R]
    Ebt = e_in.astype(np.float32).T.copy()
    S = Xq[t0s[-1]].astype(np.float32)
    for k in range(1, KSTAR + 1):
        S = _to_bf16((Ebt @ _to_bf16(S)) * Xq[t0s[-1] + k].astype(np.float32))
    csK = S.astype(np.float64).sum(axis=0)
    return csW, csK


def _run(inputs, trace=False, trace_kwargs=None):
    E, expst, expen, Xq, sh = _host_prep(inputs)

    t0s = [(L - W) * j for j in range(NCHUNK - 1)] + [T - 1 - L]

    e_in = np.ascontiguousarray((E * np.float32(1 / 128.0)).astype(NP_BF16))
    oe = np.stack([np.ones(N, np.float32), expen], axis=1)
    oe_in = np.ascontiguousarray(oe.astype(NP_BF16))

    expst_b = expst.astype(NP_BF16).astype(np.float32)

    in_maps = []
    init0 = None
    for core in range(NCORES):
        m = {"e": e_in, "oe": oe_in}
        xa = np.empty((N, L, G, FR), NP_FP8)
        for g in range(G):
            ig = np.empty((N, FR), NP_BF16)
            for c in range(K):
                j = core * G * K + g * K + c
                t0 = t0s[j]
                sl = slice(c * R, (c + 1) * R)
                xa[:, :, g, sl] = Xq[t0 + 1:t0 + L + 1].transpose(1, 0, 2)
                if j == 0:
                    i0 = (Xq[0].astype(np.float32)
                          * expst_b[:, None]).astype(NP_BF16)
                    ig[:, sl] = i0
                    init0 = i0.astype(np.float64)
                else:
                    ig[:, sl] = Xq[t0]
            m[f"i{g}"] = np.ascontiguousarray(ig)
        m["x"] = np.ascontiguousarray(xa)
        in_maps.append(m)
    cs_init0 = init0.sum(axis=0)                             # [R] f64

    nc = _build_nc()
    kw = {}
    if trace:
        kw["trace"] = True
        if trace_kwargs:
            kw.update(trace_kwargs)
    res = run_bass_kernel_spmd(nc, in_maps, core_ids=list(range(NCORES)), **kw)

    csW_host, csK_host = _host_start_colsums(e_in, Xq, t0s)

    g_log = np.zeros((NCHUNK, R), np.float64)
    for core in range(NCORES):
        outs = res.results[core]["probes"].astype(np.float64)  # [G,2,FR]
        for g in range(G):
            for c in range(K):
                j = core * G * K + g * K + c
                sl = slice(c * R, (c + 1) * R)
                csL = outs[g, 0, sl]
                if j == 0:
                    g_log[j] = np.log(csL) - np.log(cs_init0)
                elif j == NCHUNK - 1:
                    dot = outs[g, 1, sl]
                    g_log[j] = np.log(dot) - np.log(csK_host)
                else:
                    g_log[j] = np.log(csL) - np.log(csW_host[j - 1])

    z = sh.sum(axis=0) + np.log(cs_init0) + g_log.sum(axis=0)
    loss = -(z[:B] - z[B:])
    return loss.astype(np.float32), res


# revision 25
# speedup vs baseline: 1.0179x; 1.0179x over previous
"""Linear-chain CRF forward loss on 8 Trainium2 NeuronCores.

Math: per (channel, batch) row the reference runs a T=2048-step log-space
scan  alpha_t[j] = logsumexp_i(alpha_{t-1}[i] + trans[i,j]) + em_t[j]  and
returns -(z_sup - z_full).  Rewritten in linear space:

    S_k = (E'^T S_{k-1}) * X'_k        (elementwise in X')

with E' = exp(trans, forbidden->0)/128 in bf16 and X'[k][j,row] =
exp(em - sh)*128 in fp8-e4m3 (channel-0 rows masked by target), where sh is
a per-(t,row) host-side log-growth normalizer added back at the end.  The
*128//128 scaling centres X' in fp8's normal range and cancels exactly.

Sharding: T is split into 256 chunks of 8 steps (32 chains per core).
Products of positive matrices contract to rank-1 so fast that a chunk
started from the raw direction X'[t0] already telescopes correctly with NO
on-chip warm-up: the chunk-start column sums are computed on the host
(they are just colsum(X'[t0]); the pinned last chunk replays its 1 warm-up
step in emulated bf16/fp8 host arithmetic).  Per core the 32 chains run as
4 groups of 8: each round does two [128x128x512] bf16 matmuls per group
into a 2-bank PSUM tile, then the elementwise multiply is split across
three engines - DVE multiplies columns [0:672] straight out of PSUM, ACT
copies columns [672:1024] to SBUF bf16, and GPSIMD multiplies those in its
16-bit path.  End-of-chunk column sums (ones|exp(end) stationary matmul
probes) are DMA'd out and telescoped on the host in f64.
"""

import math

import numpy as np
import ml_dtypes

import concourse.bacc as bacc
import concourse.mybir as mybir
import concourse.tile as tile
from concourse.bass_utils import run_bass_kernel_spmd

B, T, N = 64, 2048, 128
R = 2 * B
NCORES = 8
G = 4                      # chain groups per core
K = 8                      # chains per group (fused width K*R = 1024)
NCHUNK = NCORES * G * K    # 256
W = 0                      # no on-chip warm-up; host computes start colsums
L = math.ceil((T + (NCHUNK - 1) * W) / NCHUNK)          # 8
KSTAR = (L - W) * (NCHUNK - 2) + L - (T - 1 - L)        # 1 (last chunk)
XTIERS = [1, 1, 2, 4]      # X prefetch tier sizes (steps); sum == L
FR = K * R                 # fused row width (1024)
HFR = FR // 2              # matmul moving-dim max is 512
ASPL = 672                 # columns multiplied by DVE direct from PSUM
BSPL = FR - ASPL           # columns via ACT copy -> GPSIMD multiply

F32 = mybir.dt.float32
BF16 = mybir.dt.bfloat16
FP8 = mybir.dt.float8e4

NP_BF16 = ml_dtypes.bfloat16
NP_FP8 = ml_dtypes.float8_e4m3

_COMPILED = {}


def _build_nc():
    if "nc" in _COMPILED:
        return _COMPILED["nc"]

    assert sum(XTIERS) == L
    nc = bacc.Bacc("TRN2", target_bir_lowering=False, debug=False,
                   num_devices=NCORES)

    e_d = nc.dram_tensor("e", [N, N], BF16, kind="ExternalInput").ap()
    oe_d = nc.dram_tensor("oe", [N, 2], BF16, kind="ExternalInput").ap()
    x_d = nc.dram_tensor("x", [N, L, G, FR], FP8,
                         kind="ExternalInput").ap()
    i_ds = [nc.dram_tensor(f"i{g}", [N, FR], BF16,
                           kind="ExternalInput").ap() for g in range(G)]
    out_d = nc.dram_tensor("probes", [G, 2, FR], F32,
                           kind="ExternalOutput").ap()

    with tile.TileContext(nc) as tc:
        with (
            tc.tile_pool(name="consts", bufs=1) as consts,
            tc.tile_pool(name="states", bufs=3) as states,
            tc.tile_pool(name="qbstage", bufs=2) as qbstage,
            tc.tile_pool(name="xtiles", bufs=1) as xtiles,
            tc.tile_pool(name="stage", bufs=1) as stage,
            tc.tile_pool(name="qpsum", bufs=1, space="PSUM") as qpsum,
        ):
            # dummy ops pull the ACT table load / GPSIMD lib load into the
            # program preamble instead of blocking the first real round
            dum = consts.tile([1, 4], BF16, tag="dum")
            nc.scalar.copy(out=dum[:, 2:4], in_=dum[:, 0:2])
            nc.gpsimd.tensor_mul(out=dum[:, 2:4], in0=dum[:, 0:2],
                                 in1=dum[:, 0:2])

            # Transfers queue in-order per HWDGE ring, so the SP ring gets
            # the small round-0 dependencies (e, inits) plus the late bulk
            # tiers, while the ACT ring moves the early X tiers in
            # parallel.  Nothing issues from the compute-critical Pool.
            xtile = []
            tier_of = []
            lo = 0
            for t, nb in enumerate(XTIERS):
                tier_of += [(t, lo)] * nb
                lo += nb

            # round-0 deps (e, inits) on the SP ring; early X tiers on the
            # ACT ring so the transfers proceed in parallel; bulk after.
            e_sb = consts.tile([N, N], BF16)
            nc.sync.dma_start(out=e_sb, in_=e_d)
            S = []
            for g in range(G):
                sg = states.tile([N, FR], BF16, tag=f"s{g}")
                nc.sync.dma_start(out=sg, in_=i_ds[g])
                S.append(sg)
            # early tiers on the ACT ring (parallel to the SP ring's inits),
            # bulk tiers behind the inits on SP
            for t in range(len(XTIERS)):
                nb = XTIERS[t]
                lo = sum(XTIERS[:t])
                xt = xtiles.tile([N, nb, G, FR], FP8, tag=f"t{t}")
                eng = nc.scalar if t < 2 else nc.sync
                eng.dma_start(out=xt, in_=x_d[:, lo:lo + nb, :, :])
                xtile.append(xt)
            oe_sb = consts.tile([N, 2], BF16)
            nc.sync.dma_start(out=oe_sb, in_=oe_d)  # needed only at k=L

            for k in range(1, L + 1):
                t, lo = tier_of[k - 1]
                for g in range(G):
                    q = qpsum.tile([N, FR], F32, tag=f"q{g}")
                    nc.tensor.matmul(q[:, :HFR], lhsT=e_sb,
                                     rhs=S[g][:, :HFR],
                                     start=True, stop=True)
                    nc.tensor.matmul(q[:, HFR:], lhsT=e_sb,
                                     rhs=S[g][:, HFR:],
                                     start=True, stop=True)
                    s_new = states.tile([N, FR], BF16, tag=f"s{g}")
                    xk = xtile[t][:, k - 1 - lo, g, :]
                    # DVE: columns [0:ASPL] straight from PSUM (2-bank span)
                    nc.vector.tensor_mul(out=s_new[:, :ASPL],
                                         in0=q[:, :ASPL], in1=xk[:, :ASPL])
                    # ACT copies [ASPL:] to bf16, GPSIMD multiplies
                    qb = qbstage.tile([N, BSPL], BF16, tag=f"qb{g}")
                    nc.scalar.copy(out=qb, in_=q[:, ASPL:])
                    nc.gpsimd.tensor_mul(out=s_new[:, ASPL:], in0=qb,
                                         in1=xk[:, ASPL:])
                    S[g] = s_new

            # end-of-chunk probes: colsum and exp(end)-dot of S(L), reusing
            # each group's PSUM banks (partitions 0:2)
            for g in range(G):
                p = qpsum.tile([N, FR], F32, tag=f"q{g}")
                nc.tensor.matmul(p[0:2, :HFR], lhsT=oe_sb,
                                 rhs=S[g][:, :HFR], start=True, stop=True)
                nc.tensor.matmul(p[0:2, HFR:], lhsT=oe_sb,
                                 rhs=S[g][:, HFR:], start=True, stop=True)
                st = stage.tile([2, FR], F32, tag=f"st{g}")
                nc.scalar.copy(out=st, in_=p[0:2, :])
                nc.sync.dma_start(out=out_d[g], in_=st)

    nc.compile()
    _COMPILED["nc"] = nc
    return nc


def _host_prep(inputs):
    em = np.asarray(inputs["emissions"], np.float32)
    tgt = np.asarray(inputs["target"])
    trans = np.asarray(inputs["transitions"], np.float32)
    st = np.asarray(inputs["start_transitions"], np.float32)
    en = np.asarray(inputs["end_transitions"], np.float32)
    ft = np.asarray(inputs["forbidden_transitions"]).astype(bool)
    sft = np.asarray(inputs["start_forbidden_transitions"]).astype(bool)
    eft = np.asarray(inputs["end_forbidden_transitions"]).astype(bool)
    mask = np.asarray(inputs["mask"]).astype(bool)
    assert mask.all(), "kernel specialized for all-true mask"

    E = np.where(ft, 0.0, np.exp(trans)).astype(np.float32)
    expst = np.where(sft, 0.0, np.exp(st)).astype(np.float32)
    expen = np.where(eft, 0.0, np.exp(en)).astype(np.float32)

    x1 = np.exp(em.astype(np.float32)).transpose(1, 2, 0)    # [T,N,B]
    x0 = x1 * tgt.astype(np.float32).transpose(1, 2, 0)
    X = np.concatenate([x0, x1], axis=2)                     # [T,N,R]

    Ebar = np.float64(E.astype(np.float64).mean())
    sh = np.log(np.maximum(X.sum(axis=1, dtype=np.float64) * Ebar, 1e-300))
    Xs = (X * (np.exp(-sh)[:, None, :] * 128.0)).astype(np.float32)
    Xq = np.minimum(Xs, np.float32(240.0)).astype(NP_FP8)    # [T,N,R] fp8
    return E, expst, expen, Xq, sh


def kernel(**inputs):
    loss, _ = _run(inputs)
    return loss


def _to_bf16(x):
    """Round-to-nearest-even f32 -> bf16 -> f32 (matches device rounding)."""
    u = np.ascontiguousarray(x, np.float32).view(np.uint32)
    r = (u + 0x7FFF + ((u >> 16) & 1)) & 0xFFFF0000
    return r.view(np.float32)


def _host_start_colsums(e_in, Xq, t0s):
    """Chunk-start column sums, host side.  Mid chunks start at the raw
    direction X'[t0] (W=0), so the start colsum is just its column sum.
    The pinned last chunk replays its KSTAR warm-up steps in emulated
    device arithmetic (bf16 matmul inputs, f32 accumulate, bf16 state)."""
    csW = np.stack([Xq[t0s[j]].astype(np.float64).sum(axis=0)
                    for j in range(1, NCHUNK - 1)])          # [NCHUNK-2, R]
    Ebt = e_in.astype(np.float32).T.copy()
    S = Xq[t0s[-1]].astype(np.float32)
    for k in range(1, KSTAR + 1):
        S = _to_bf16((Ebt @ _to_bf16(S)) * Xq[t0s[-1] + k].astype(np.float32))
    csK = S.astype(np.float64).sum(axis=0)
    return csW, csK


def _run(inputs, trace=False, trace_kwargs=None):
    E, expst, expen, Xq, sh = _host_prep(inputs)

    t0s = [(L - W) * j for j in range(NCHUNK - 1)] + [T - 1 - L]

    e_in = np.ascontiguousarray((E * np.float32(1 / 128.0)).astype(NP_BF16))
    oe = np.stack([np.ones(N, np.float32), expen], axis=1)
    oe_in = np.ascontiguousarray(oe.astype(NP_BF16))

    expst_b = expst.astype(NP_BF16).astype(np.float32)

    in_maps = []
    init0 = None
    for core in range(NCORES):
        m = {"e": e_in, "oe": oe_in}
        xa = np.empty((N, L, G, FR), NP_FP8)
        for g in range(G):
            ig = np.empty((N, FR), NP_BF16)
            for c in range(K):
                j = core * G * K + g * K + c
                t0 = t0s[j]
                sl = slice(c * R, (c + 1) * R)
                xa[:, :, g, sl] = Xq[t0 + 1:t0 + L + 1].transpose(1, 0, 2)
                if j == 0:
                    i0 = (Xq[0].astype(np.float32)
                          * expst_b[:, None]).astype(NP_BF16)
                    ig[:, sl] = i0
                    init0 = i0.astype(np.float64)
                else:
                    ig[:, sl] = Xq[t0]
            m[f"i{g}"] = np.ascontiguousarray(ig)
        m["x"] = np.ascontiguousarray(xa)
        in_maps.append(m)
    cs_init0 = init0.sum(axis=0)                             # [R] f64

    nc = _build_nc()
    kw = {}
    if trace:
        kw["trace"] = True
        if trace_kwargs:
            kw.update(trace_kwargs)
    res = run_bass_kernel_spmd(nc, in_maps, core_ids=list(range(NCORES)), **kw)

    csW_host, csK_host = _host_start_colsums(e_in, Xq, t0s)

    g_log = np.zeros((NCHUNK, R), np.float64)
    for core in range(NCORES):
        outs = res.results[core]["probes"].astype(np.float64)  # [G,2,FR]
        for g in range(G):
            for c in range(K):
                j = core * G * K + g * K + c
                sl = slice(c * R, (c + 1) * R)
                csL = outs[g, 0, sl]
                if j == 0:
                    g_log[j] = np.log(csL) - np.log(cs_init0)
                elif j == NCHUNK - 1:
                    dot = outs[g, 1, sl]
                    g_log[j] = np.log(dot) - np.log(csK_host)
                else:
                    g_log[j] = np.log(csL) - np.log(csW_host[j - 1])

    z = sh.sum(axis=0) + np.log(cs_init0) + g_log.sum(axis=0)
    loss = -(z[:B] - z[B:])
    return loss.astype(np.float32), res
